# revision 39
# baseline (speedup 1.0000x reference)
"""CBAM (channel+spatial attention) + residual, data-parallel over 8 NeuronCores.

Math per batch item (C=256, H=W=56):
  ca   = sigmoid(mlp(mean_hw(x)) + mlp(max_hw(x)))        # (C,)   channel attn
  xc   = x * ca[:, None, None]
  pool = stack(mean_c(xc), max_c(xc))                     # (2, H, W)
  sa   = sigmoid(conv7x7(pool))                           # (H, W) spatial attn
  out  = x + xc * sa  =  x * (1 + ca ⊗ sa)

Per-core layout (B_local=4): channels on SBUF partitions, 2 tiles of
[128, 3136] per image.  hw-stats via ACT accum (sum) + DVE reduce (max);
MLP + channel-mean + 7x7 conv + rank-1 (1+ca⊗sa) factor on PE (fp32r);
channel-max via GPSIMD partition-axis (C) reduce; final fused
(F+1)*x on DVE scalar_tensor_tensor.  Conv is done as a K=56 matmul over
(pair-of-b x stat x ky) rows with block-diagonal weights, accumulated
over kx with shifted views of a zero-padded (62x62) pooled map.
"""

import sys

try:  # the axon site dir already carries the repo; /opt is the fallback
    import concourse  # noqa: F401
except ImportError:
    sys.path.insert(0, "/opt/trn_rl_repo")

import ml_dtypes
import numpy as np

import concourse.bass as bass
import concourse.bacc as bacc
import concourse.mybir as mybir
import concourse.tile as tile
from concourse.bass_utils import run_bass_kernel_spmd

F32 = mybir.dt.float32
BF16 = mybir.dt.bfloat16

M = 8          # cores
B, C, H, W = 32, 256, 56, 56
BL = B // M    # batch per core
CT = 2         # channel tiles of 128
P = 128
HW = H * W     # 3136
R = 16         # reduced channels
KK = 7         # conv kernel
PADW = W + KK - 1          # 62
FLAT = PADW * PADW         # 3844
P7W = (H - 1) * PADW + W + KK - 1   # 3472: max j = 55*62+55+6
NCH = 7                    # mean/conv output row-chunks (8 rows each)
ROWS = H // NCH            # 8 rows per chunk
CHW = ROWS * W             # 448 elements per chunk


def build_nc():
    nc = bacc.Bacc(target_bir_lowering=False, num_devices=M)

    x_d = nc.dram_tensor("x", [BL, C, H, W], F32, kind="ExternalInput").ap()
    out_d = nc.dram_tensor("out", [BL, C, H, W], F32, kind="ExternalOutput").ap()
    w1t_d = nc.dram_tensor("w1t", [P, CT, R], F32, kind="ExternalInput").ap()
    w2t_d = nc.dram_tensor("w2t", [R, C], F32, kind="ExternalInput").ap()
    wc_d = nc.dram_tensor("wconv", [2 * 2 * KK, KK, 2], BF16, kind="ExternalInput").ap()

    xv = x_d.rearrange("b (t p) h w -> b t p (h w)", t=CT)
    ov = out_d.rearrange("b (t p) h w -> b t p (h w)", t=CT)

    with tile.TileContext(nc) as tc:
        with (
            tc.tile_pool(name="xt", bufs=1) as xt_pool,
            tc.tile_pool(name="xc", bufs=1) as xc_pool,
            tc.tile_pool(name="pv", bufs=2) as pv_pool,
            tc.tile_pool(name="p7", bufs=2) as p7_pool,
            tc.tile_pool(name="sa", bufs=2) as sa_pool,
            tc.tile_pool(name="st", bufs=1) as st_pool,
            tc.tile_pool(name="singles", bufs=1) as singles,
            tc.tile_pool(name="fl", bufs=2, space="DRAM") as fl_pool,
            tc.tile_pool(name="psmall", bufs=2, space="PSUM") as psum_small,
            tc.tile_pool(name="pF", bufs=2, space="PSUM") as psum_f,
        ):
            # ---- constants / weights ----
            w1t = singles.tile([P, CT, R], F32, name="w1t", tag="w1t")
            w2t = singles.tile([R, C], F32, name="w2t", tag="w2t")
            wc = singles.tile([2 * 2 * KK, KK, 2], BF16, name="wc", tag="wc")
            zpad = singles.tile([2, 384], BF16, name="zpad", tag="zpad")
            nc.sync.dma_start(out=w1t, in_=w1t_d)
            nc.sync.dma_start(out=w2t, in_=w2t_d)
            nc.sync.dma_start(out=wc, in_=wc_d)
            nc.vector.memset(zpad, 0.0)

            # channel attention, two layouts:
            #  caZ[i, b, c] = ca[b, c] when i == b%2 else 0 — the zero row
            #  makes a legal K=2 rank-1 matmul against the 2-row sa tile.
            #  cac[p, t, b] — column layout for per-partition scales.
            caT4 = singles.tile([1, BL, C], BF16, name="caT4", tag="caT4")
            caZ = singles.tile([2, BL, C], BF16, name="caZ", tag="caZ")
            cac = singles.tile([P, CT, BL], F32, name="cac", tag="cac")
            h_sb = singles.tile([R, 2 * BL], F32, name="h", tag="h")

            # ---- loads ----
            xt = [[xt_pool.tile([P, HW], F32, name=f"xt{b}{t}", tag=f"xt{b}{t}") for t in range(CT)]
                  for b in range(BL)]
            for b in range(BL):
                for t in range(CT):
                    nc.sync.dma_start(out=xt[b][t], in_=xv[b, t])

            st = [st_pool.tile([P, 2 * BL], F32, name=f"st{t}", tag=f"st{t}")
                  for t in range(CT)]
            # caZ zero rows written once, rows scattered per-b below
            nc.vector.memset(caZ, 0.0)

            # ---- per-image spatial pooling + conv + final ----
            fl = [None] * BL
            sa_p = [None] * (BL // 2)
            for b in range(BL):
                # hw stats for this image: sum via ACT accumulate (in-place
                # copy), max via DVE reduce
                for t in range(CT):
                    nc.scalar.activation(
                        out=xt[b][t], in_=xt[b][t],
                        func=mybir.ActivationFunctionType.Copy,
                        accum_out=st[t][:, b:b + 1],
                    )
                    nc.vector.tensor_reduce(
                        out=st[t][:, BL + b:BL + b + 1], in_=xt[b][t],
                        axis=mybir.AxisListType.X, op=mybir.AluOpType.max,
                    )
                # per-image channel-attention MLP (avoids an all-b barrier)
                stv = [st[t].rearrange("p (s bb) -> p s bb", bb=BL) for t in range(CT)]
                nc.vector.tensor_scalar_mul(
                    out=stv[0][:, 0:1, b], in0=stv[0][:, 0:1, b], scalar1=1.0 / HW)
                nc.vector.tensor_scalar_mul(
                    out=stv[1][:, 0:1, b], in0=stv[1][:, 0:1, b], scalar1=1.0 / HW)
                ph = psum_small.tile([R, 2], F32, name="ph", tag="sm")
                for t in range(CT):
                    nc.tensor.matmul(ph, lhsT=w1t[:, t, :], rhs=stv[t][:, :, b],
                                     start=(t == 0), stop=(t == CT - 1))
                nc.scalar.activation(out=h_sb[:, 2 * b:2 * b + 2], in_=ph,
                                     func=mybir.ActivationFunctionType.Relu)
                nc.vector.tensor_tensor(
                    out=h_sb[:, 2 * b:2 * b + 1], in0=h_sb[:, 2 * b:2 * b + 1],
                    in1=h_sb[:, 2 * b + 1:2 * b + 2], op=mybir.AluOpType.add)
                hb = h_sb[:, 2 * b:2 * b + 1]
                pca = psum_small.tile([1, C], F32, name="pca", tag="sm")
                nc.tensor.matmul(pca, lhsT=hb, rhs=w2t, start=True, stop=True)
                nc.scalar.activation(out=caT4[0:1, b, :], in_=pca,
                                     func=mybir.ActivationFunctionType.Sigmoid)
                nc.sync.dma_start(out=caZ[b % 2:b % 2 + 1, b, :],
                                  in_=caT4[0:1, b, :])
                for t in range(CT):
                    pcc = psum_small.tile([P, 1], F32, name="pcc", tag="sm")
                    nc.tensor.matmul(pcc, lhsT=w2t[:, t * P:(t + 1) * P], rhs=hb,
                                     start=True, stop=True)
                    nc.scalar.activation(out=cac[:, t, b:b + 1], in_=pcc,
                                         func=mybir.ActivationFunctionType.Sigmoid)

                # xc = x * ca  (ACT, per-partition scale)
                xcb = [xc_pool.tile([P, HW], BF16, name=f"xc{b}{t}", tag=f"xc{b % 2}{t}")
                       for t in range(CT)]
                for t in range(CT):
                    nc.vector.tensor_scalar_mul(
                        out=xcb[t], in0=xt[b][t],
                        scalar1=cac[:, t, b:b + 1])

                # pooled stats: DVE pre-combines the ct pair (bf16 2x),
                # GPSIMD does one partition-axis reduce per stat.
                # pv planes: 0 = avg (channel sum; 1/C folded into wconv), 1 = max.
                xm = xc_pool.tile([P, HW], BF16, name=f"xm{b % 2}", tag=f"xm{b % 2}", bufs=1)
                xs = xc_pool.tile([P, HW], BF16, name=f"xs{b % 2}", tag=f"xs{b % 2}", bufs=1)
                nc.vector.tensor_tensor(out=xm, in0=xcb[0], in1=xcb[1],
                                        op=mybir.AluOpType.max)
                nc.vector.tensor_tensor(out=xs, in0=xcb[0], in1=xcb[1],
                                        op=mybir.AluOpType.add)
                pv = pv_pool.tile([1, 2, HW], BF16, name="pv", tag="pv")
                nc.gpsimd.tensor_reduce(
                    out=pv[0:1, 1, :], in_=xm,
                    axis=mybir.AxisListType.C, op=mybir.AluOpType.max)
                # bf16 rounding of the pooled planes only feeds the 7x7
                # conv input; fp32-accumulated internally.
                with nc.allow_low_precision(reason="pooled maps are bf16"):
                    nc.gpsimd.tensor_reduce(
                        out=pv[0:1, 0, :], in_=xs,
                        axis=mybir.AxisListType.C, op=mybir.AluOpType.add)

                # zero-padded (2, 62, 62) pooled map in DRAM
                flb = fl_pool.tile([2, FLAT], BF16, name=f"fl{b % 2}", tag=f"fl{b % 2}")
                fl[b] = flb
                flv = flb.rearrange("s (r w) -> s r w", w=PADW)
                # borders: prefix, suffix, and the wrap-around middle strip
                nc.sync.dma_start(out=flb[:, 0:189], in_=zpad[:, 0:189])
                nc.sync.dma_start(out=flb[:, FLAT - 189:FLAT],
                                  in_=zpad[:, 0:189])
                mid = flb[:, 245:245 + 55 * PADW].rearrange(
                    "s (r w) -> s r w", w=PADW)[:, :, 0:6]
                nc.sync.dma_start(
                    out=mid,
                    in_=zpad[:, 0:330].rearrange("s (r w) -> s r w", w=6))
                # both pooled planes (rows 0 and 32) -> zero-padded interior
                nc.sync.dma_start(
                    out=flv[:, 3:3 + H, 3:3 + W],
                    in_=pv[0:1, 0:2, :].rearrange("q s (h w) -> q s h w", w=W))

            # ---- convs per pair, then all rank-1 finals + stores ----
            for pair in range(BL // 2):
                # im2col-lite: rows (i, s, ky) = fl[2*pair+i][s, 62*ky:+3472]
                p7 = p7_pool.tile([4 * KK, P7W], BF16, name=f"p7{pair}", tag="p7")
                for i in range(2):
                    src = fl[2 * pair + i]
                    nc.sync.dma_start(
                        out=p7[i * 2 * KK:(i + 1) * 2 * KK, :],
                        in_=bass.AP(tensor=src.tensor, offset=src.offset,
                                    ap=[src.ap[0], [PADW, KK], [1, P7W]]))
                # conv: accumulate 7 kx-shifted K=56 matmuls per row-chunk
                sp = sa_pool.tile([2, HW], BF16, name=f"sa{pair}", tag="sa")
                sa_p[pair] = sp
                for k in range(NCH):
                    pc = psum_small.tile([2, CHW], F32, name="pconv", tag="sm")
                    for kx in range(KK):
                        rhs = bass.AP(
                            tensor=p7.tensor,
                            offset=p7.offset + PADW * ROWS * k + kx,
                            ap=[p7.ap[0], [PADW, ROWS], [1, W]])
                        nc.tensor.matmul(pc, lhsT=wc[:, kx, :], rhs=rhs,
                                         start=(kx == 0), stop=(kx == KK - 1))
                    nc.scalar.activation(
                        out=sp[:, k * CHW:(k + 1) * CHW], in_=pc,
                        func=mybir.ActivationFunctionType.Sigmoid)

            # final: out = x * (1 + ca (x) sa).  K=2 rank-1 matmul against
            # both sa rows; caZ's zero row kills the other one.
            for bb in range(BL):
                sp = sa_p[bb // 2]
                for t in range(CT):
                    for c0 in range(0, HW, 1536):
                        cw = min(1536, HW - c0)
                        pf = psum_f.tile([P, 1536], F32, name="pF", tag="pF")
                        for s0 in range(0, cw, 512):
                            sw = min(512, cw - s0)
                            nc.tensor.matmul(
                                pf[:, s0:s0 + sw],
                                lhsT=caZ[:, bb, t * P:(t + 1) * P],
                                rhs=sp[:, c0 + s0:c0 + s0 + sw],
                                start=True, stop=True)
                        nc.vector.scalar_tensor_tensor(
                            out=xt[bb][t][:, c0:c0 + cw],
                            in0=pf[:, 0:cw], scalar=1.0,
                            op0=mybir.AluOpType.add,
                            in1=xt[bb][t][:, c0:c0 + cw],
                            op1=mybir.AluOpType.mult)
                    nc.sync.dma_start(out=ov[bb, t], in_=xt[bb][t])
    nc.compile()
    return nc


_NC = None


def _get_nc():
    global _NC
    if _NC is None:
        _NC = build_nc()
    return _NC


def _prep_weights(w1, w2, w_sp):
    w1t = np.ascontiguousarray(
        w1.T.reshape(CT, P, R).transpose(1, 0, 2)).astype(np.float32)
    w2t = np.ascontiguousarray(w2.T).astype(np.float32)
    wc = np.zeros((2 * 2 * KK, KK, 2), np.float32)
    for i in range(2):
        for s in range(2):
            scale = (1.0 / C) if s == 0 else 1.0
            for ky in range(KK):
                wc[i * 2 * KK + s * KK + ky, :, i] = w_sp[0, s, ky, :] * scale
    return w1t, w2t, wc.astype(ml_dtypes.bfloat16)


def kernel(x, w1, w2, w_sp):
    x = np.asarray(x, dtype=np.float32)
    w1t, w2t, wc = _prep_weights(np.asarray(w1), np.asarray(w2), np.asarray(w_sp))
    nc = _get_nc()
    in_maps = [
        {"x": np.ascontiguousarray(x[i * BL:(i + 1) * BL]),
         "w1t": w1t, "w2t": w2t, "wconv": wc}
        for i in range(M)
    ]
    try:
        res = run_bass_kernel_spmd(nc, in_maps, list(range(M)))
    except Exception:
        # one retry: a wedged NeuronCore from a prior run surfaces as a
        # transient NRT execution error on the first dispatch
        res = run_bass_kernel_spmd(nc, in_maps, list(range(M)))
    return np.concatenate([res.results[i]["out"] for i in range(M)], axis=0)
